# revision 15
# baseline (speedup 1.0000x reference)
"""MLA (multi-head latent attention) prefill kernel for Trainium2, 8 NeuronCores.

Sharding: 8 cores = 2 batches x 4 head-groups (4 heads each). Each core
computes a partial output (its heads' contribution through Wo); host sums
the 4 partials per batch.

Per-core dataflow (feature-major "transposed" layout so weights serve
directly as PE stationary operands):
  x -> xT (PE transpose) -> c_qT/c_kvT (down proj) -> qT/kT (up proj,
  + rope computed in natural layout then transposed) ; v in natural layout.
  Scores computed transposed sT[k,q] = kT.T @ qT, causal-tiled; exp on ACT;
  row sums via DVE accumulate + ones-matmul partition reduce; av as
  avT[d,q] = v.T @ sExpT; normalize; out = attnT.T @ Wo_slice.
All matmuls bf16 with fp32 PSUM accumulation.
"""

import numpy as np
from contextlib import ExitStack

import concourse.bass as bass
import concourse.mybir as mybir
import concourse.tile as tile
from concourse import bacc
from concourse.masks import make_identity
from concourse.bass_utils import run_bass_kernel_spmd

P = 128
S = 2048          # sequence length
D = 2048          # model dim
DC = 512          # latent dim
H = 4             # heads per core
DH = 128          # head dim (nope)
DR = 64           # rope dim per head
T = 512           # token tile
NT = S // T       # 4 token tiles
KC = D // P       # 16 contraction chunks over model dim
NKT = S // P      # 16 key tiles
GQ = T // P       # 4 q-subtiles per token tile
SCALE = 1.0 / float(np.sqrt(DH))

F32 = mybir.dt.float32
BF16 = mybir.dt.bfloat16

_CACHE = {}


def _build():
    nc = bacc.Bacc(None, target_bir_lowering=False)

    # ---- DRAM I/O (per-core shard) ----
    x_d = nc.dram_tensor("x_b", [S, D], F32, kind="ExternalInput")
    cos_d = nc.dram_tensor("freqs_cos", [S, DR // 2], F32, kind="ExternalInput")
    sin_d = nc.dram_tensor("freqs_sin", [S, DR // 2], F32, kind="ExternalInput")
    wqd_d = nc.dram_tensor("Wq_down", [D, DC], F32, kind="ExternalInput")
    wkvd_d = nc.dram_tensor("Wkv_down", [D, DC], F32, kind="ExternalInput")
    wqu_d = nc.dram_tensor("Wq_up_s", [DC, H * DH], F32, kind="ExternalInput")
    wku_d = nc.dram_tensor("Wk_up_s", [DC, H * DH], F32, kind="ExternalInput")
    wvu_d = nc.dram_tensor("Wv_up_s", [DC, H * DH], F32, kind="ExternalInput")
    wqr_d = nc.dram_tensor("Wq_rope_s", [D, H * DR], F32, kind="ExternalInput")
    wkr_d = nc.dram_tensor("Wk_rope_s", [D, H * DR], F32, kind="ExternalInput")
    wo_d = nc.dram_tensor("Wo_s", [H * DH, D], F32, kind="ExternalInput")
    out_d = nc.dram_tensor("out_partial", [S, D], F32, kind="ExternalOutput")

    with tile.TileContext(nc) as tc, ExitStack() as top:
        const = top.enter_context(tc.tile_pool(name="const", bufs=1))
        actp = top.enter_context(tc.tile_pool(name="actp", bufs=1))

        # ---- constants ----
        ident = const.tile([P, P], BF16, tag="ident")
        make_identity(nc, ident[:])
        ones_col = const.tile([P, 1], BF16, tag="ones_col")
        nc.vector.memset(ones_col[:], 1.0)
        ones_colf = const.tile([P, 1], F32, tag="ones_colf")
        nc.vector.memset(ones_colf[:], 1.0)
        ones_row = const.tile([1, P], F32, tag="ones_row")
        nc.vector.memset(ones_row[:], 1.0)
        cos_sb = const.tile([P, NKT, DR // 2], F32, tag="cos")
        sin_sb = const.tile([P, NKT, DR // 2], F32, tag="sin")
        nc.sync.dma_start(cos_sb[:], cos_d.ap().rearrange("(n p) f -> p n f", p=P))
        nc.sync.dma_start(sin_sb[:], sin_d.ap().rearrange("(n p) f -> p n f", p=P))
        masks = []
        for i in range(4):
            m = const.tile([P, T], BF16, tag=f"mask{i}", name=f"mask{i}")
            nc.gpsimd.memset(m[:], 0.0)
            # keep (0) where qcol >= p + 128*i, else -30000
            nc.gpsimd.affine_select(
                out=m[:], in_=m[:], compare_op=mybir.AluOpType.is_ge,
                fill=-30000.0, base=-(P * i), channel_multiplier=-1,
                pattern=[[1, T]],
            )
            masks.append(m)

        # ---- persistent activations ----
        q_nT = actp.tile([P, H, S], BF16, tag="q_nT")
        k_nT = actp.tile([P, H, S], BF16, tag="k_nT")
        q_rT = actp.tile([P, 2, S], BF16, tag="q_rT")   # [ (h%2)*64+r, h//2, t ]
        k_rT = actp.tile([P, 2, S], BF16, tag="k_rT")
        v_sb = actp.tile([P, NKT, H, DH], BF16, tag="v_sb")

        # ================= Phase 1+2+3: projections =================
        with ExitStack() as proj:
            wts = proj.enter_context(tc.tile_pool(name="wts", bufs=1))
            stg = proj.enter_context(tc.tile_pool(name="stg", bufs=2))
            ptr = proj.enter_context(tc.tile_pool(name="ptr", bufs=1))
            pps = proj.enter_context(tc.tile_pool(name="pps", bufs=1, space="PSUM"))

            # ---- load + cast weights (except Wo) ----
            w_dn = wts.tile([P, KC, 2 * DC], BF16, tag="w_dn")     # [:, :, :512]=q, [:, :, 512:]=kv
            w_rp = wts.tile([P, KC, 2 * H * DR], BF16, tag="w_rp")  # q ropes | k ropes
            w_qu = wts.tile([P, DC // P, H * DH], BF16, tag="w_qu")
            w_ku = wts.tile([P, DC // P, H * DH], BF16, tag="w_ku")
            w_vu = wts.tile([P, DC // P, H * DH], BF16, tag="w_vu")

            def load_x_sub(j, s4, xT):
                # load 128 x rows, cast to bf16, PE-transpose into xT chunks
                st = stg.tile([P, D], F32, tag="stage", name="xst")
                nc.sync.dma_start(st[:], x_d.ap()[(j * GQ + s4) * P:(j * GQ + s4 + 1) * P, :])
                xbf = ptr.tile([P, D], BF16, tag="xbf", bufs=2, name="xbf")
                nc.vector.tensor_copy(xbf[:], st[:])
                for k in range(KC):
                    tp = pps.tile([P, P], BF16, tag="tp", bufs=2, name="tp")
                    nc.tensor.transpose(tp[:], xbf[:, k * P:(k + 1) * P], ident[:])
                    nc.scalar.copy(xT[:, k, s4 * P:(s4 + 1) * P], tp[:])

            xT0 = ptr.tile([P, KC, T], BF16, tag="xT", bufs=2, name="xT0")
            for s4 in range(GQ):
                load_x_sub(0, s4, xT0)

            wqd_r = wqd_d.ap().rearrange("(kc p) c -> p kc c", p=P)
            wkvd_r = wkvd_d.ap().rearrange("(kc p) c -> p kc c", p=P)
            for kc4 in range(0, KC, 4):
                for wsrc, dst_off in ((wqd_r, 0), (wkvd_r, DC)):
                    st = stg.tile([P, 4, DC], F32, tag="stage", name="wst")
                    nc.sync.dma_start(st[:], wsrc[:, kc4:kc4 + 4, :])
                    nc.vector.tensor_copy(
                        w_dn[:, kc4:kc4 + 4, dst_off:dst_off + DC], st[:])
            wqr_r = wqr_d.ap().rearrange("(kc p) r -> p kc r", p=P)
            wkr_r = wkr_d.ap().rearrange("(kc p) r -> p kc r", p=P)
            for kc8 in range(0, KC, 8):
                for wsrc, dst_off in ((wqr_r, 0), (wkr_r, H * DR)):
                    st = stg.tile([P, 8, H * DR], F32, tag="stage", name="wst2")
                    nc.sync.dma_start(st[:], wsrc[:, kc8:kc8 + 8, :])
                    nc.vector.tensor_copy(
                        w_rp[:, kc8:kc8 + 8, dst_off:dst_off + H * DR], st[:])
            for wd, wt in ((wqu_d, w_qu), (wku_d, w_ku), (wvu_d, w_vu)):
                st = stg.tile([P, DC // P, H * DH], F32, tag="stage", name="wst3")
                nc.sync.dma_start(st[:], wd.ap().rearrange("(cc p) d -> p cc d", p=P))
                nc.vector.tensor_copy(wt[:], st[:])

            xT = xT0
            for j in range(NT):  # token tiles
                xT_next = (ptr.tile([P, KC, T], BF16, tag="xT", bufs=2, name="xTn")
                           if j + 1 < NT else None)

                # ---- P2a: down projections c_qT, c_kvT ----
                # next tile's transposes interleaved between MM blocks (HAM warmth)
                c_qT = ptr.tile([P, DC // P, T], BF16, tag="c_qT", name="c_qT")
                c_kvT = ptr.tile([P, DC // P, T], BF16, tag="c_kvT", name="c_kvT")
                blk = 0
                for fam, (dst, woff) in enumerate(((c_qT, 0), (c_kvT, DC))):
                    for half in range(2):
                        pa = pps.tile([P, T], F32, tag="acc", bufs=4, name="pa")
                        pb = pps.tile([P, T], F32, tag="acc", bufs=4, name="pb")
                        for k in range(KC):
                            nc.tensor.matmul(
                                pa[:], w_dn[:, k, woff + half * 256: woff + half * 256 + P],
                                xT[:, k, :], start=(k == 0), stop=(k == KC - 1))
                            nc.tensor.matmul(
                                pb[:], w_dn[:, k, woff + half * 256 + P: woff + half * 256 + 2 * P],
                                xT[:, k, :], start=(k == 0), stop=(k == KC - 1))
                        nc.vector.tensor_copy(dst[:, 2 * half, :], pa[:])
                        nc.vector.tensor_copy(dst[:, 2 * half + 1, :], pb[:])
                        if xT_next is not None:
                            load_x_sub(j + 1, blk, xT_next)
                        blk += 1

                # ---- P2b: rope projections (natural layout), apply rope, transpose ----
                for s4 in range(GQ):
                    pr = pps.tile([P, T], F32, tag="acc", bufs=4, name="pr")
                    for k in range(KC):
                        nc.tensor.matmul(
                            pr[:], xT[:, k, s4 * P:(s4 + 1) * P], w_rp[:, k, :],
                            start=(k == 0), stop=(k == KC - 1))
                    # cos/sin replicated x8 across (2 fams x 4 heads)
                    crep = ptr.tile([P, 8, DR // 2], F32, tag="crep", bufs=2, name="crep")
                    srep = ptr.tile([P, 8, DR // 2], F32, tag="srep", bufs=2, name="srep")
                    for r in range(8):
                        nc.scalar.copy(crep[:, r, :], cos_sb[:, j * GQ + s4, :])
                        nc.scalar.copy(srep[:, r, :], sin_sb[:, j * GQ + s4, :])
                    # rope apply: out_r = xr*c - xi*s ; out_i = xr*s + xi*c
                    xr = pr[:, 0:2 * H * DR:2]
                    xi = pr[:, 1:2 * H * DR:2]
                    t1 = ptr.tile([P, H * DR], F32, tag="t1", bufs=2, name="t1")
                    t2 = ptr.tile([P, H * DR], F32, tag="t2", bufs=2, name="t2")
                    rap = ptr.tile([P, 2 * H * DR], BF16, tag="rap", bufs=2, name="rap")
                    nc.vector.tensor_mul(t1[:], xr, crep[:].rearrange("p a b -> p (a b)"))
                    nc.vector.tensor_mul(t2[:], xi, srep[:].rearrange("p a b -> p (a b)"))
                    nc.vector.tensor_sub(rap[:, 0:2 * H * DR:2], t1[:], t2[:])
                    nc.vector.tensor_mul(t1[:], xr, srep[:].rearrange("p a b -> p (a b)"))
                    nc.vector.tensor_mul(t2[:], xi, crep[:].rearrange("p a b -> p (a b)"))
                    nc.vector.tensor_add(rap[:, 1:2 * H * DR:2], t1[:], t2[:])
                    # transpose rope cols to ropeT layout
                    for rc in range(4):
                        tp = pps.tile([P, P], BF16, tag="tp", bufs=2, name="tpr")
                        nc.tensor.transpose(tp[:], rap[:, rc * P:(rc + 1) * P], ident[:])
                        dst = q_rT if rc < 2 else k_rT
                        nc.scalar.copy(
                            dst[:, rc % 2, (j * GQ + s4) * P:(j * GQ + s4 + 1) * P], tp[:])

                # ---- P3: up projections qT, kT (transposed), v (natural) ----
                for h in range(H):
                    pq = pps.tile([P, T], F32, tag="acc", bufs=4, name="pq")
                    pk = pps.tile([P, T], F32, tag="acc", bufs=4, name="pk")
                    for cc in range(DC // P):
                        nc.tensor.matmul(pq[:], w_qu[:, cc, h * DH:(h + 1) * DH],
                                         c_qT[:, cc, :], start=(cc == 0), stop=(cc == 3))
                        nc.tensor.matmul(pk[:], w_ku[:, cc, h * DH:(h + 1) * DH],
                                         c_kvT[:, cc, :], start=(cc == 0), stop=(cc == 3))
                    nc.vector.tensor_copy(q_nT[:, h, j * T:(j + 1) * T], pq[:])
                    nc.vector.tensor_copy(k_nT[:, h, j * T:(j + 1) * T], pk[:])
                for s4 in range(GQ):
                    pv = pps.tile([P, H * DH], F32, tag="acc", bufs=4, name="pv")
                    for cc in range(DC // P):
                        nc.tensor.matmul(pv[:], c_kvT[:, cc, s4 * P:(s4 + 1) * P],
                                         w_vu[:, cc, :], start=(cc == 0), stop=(cc == 3))
                    for h in range(H):
                        nc.vector.tensor_copy(
                            v_sb[:, j * GQ + s4, h, :], pv[:, h * DH:(h + 1) * DH])
                xT = xT_next

        # ================= Phase 4: attention =================
        attnT = top.enter_context(tc.tile_pool(name="attnTp", bufs=1)).tile(
            [P, H, S], BF16, tag="attnT")
        with ExitStack() as att:
            atr = att.enter_context(tc.tile_pool(name="atr", bufs=1))
            wop = att.enter_context(tc.tile_pool(name="wop", bufs=1))
            ostg = att.enter_context(tc.tile_pool(name="ostg", bufs=2))

            # Wo load + cast (overlaps with attention start)
            wo_sb = wop.tile([P, H, D], BF16, tag="wo_sb")
            wo_r = wo_d.ap().rearrange("(hh p) dd -> p hh dd", p=P)
            for h2 in range(0, H, 2):
                st = ostg.tile([P, 2, D], F32, tag="ostage", name="wost")
                nc.sync.dma_start(st[:], wo_r[:, h2:h2 + 2, :])
                nc.vector.tensor_copy(wo_sb[:, h2:h2 + 2, :], st[:])

            att2 = ExitStack()
            aps = att2.enter_context(tc.tile_pool(name="aps", bufs=1, space="PSUM"))
            for g in range(S // T):
                for hp in range(2):
                    heads = (2 * hp, 2 * hp + 1)
                    pavs = [aps.tile([P, T], F32, tag="pav", bufs=2, name="pav")
                            for _ in range(2)]
                    saccs = [atr.tile([P, T], F32, tag="sacc", bufs=4, name="sacc")
                             for _ in range(2)]
                    nkt = GQ * g + GQ
                    for kk in range(nkt):
                        # diagonal narrowing: cols < m are fully masked
                        di = kk - GQ * g
                        m = max(di, 0) * P
                        qs = slice(g * T + m, (g + 1) * T)
                        psts = [aps.tile([P, T], F32, tag="pst", bufs=4, name="pst")
                                for _ in range(2)]
                        for i, h in enumerate(heads):
                            nc.tensor.matmul(
                                psts[i][:, m:], k_nT[:, h, kk * P:(kk + 1) * P],
                                q_nT[:, h, qs], start=True, stop=False)
                        for i, h in enumerate(heads):
                            rb = i * DR
                            nc.tensor.matmul(
                                psts[i][:, m:], k_rT[rb:rb + DR, hp, kk * P:(kk + 1) * P],
                                q_rT[rb:rb + DR, hp, qs],
                                start=False, stop=True)
                        sEs = []
                        for i, h in enumerate(heads):
                            if di >= 0:
                                nc.vector.tensor_add(
                                    psts[i][:, m:], psts[i][:, m:], masks[di][:, m:])
                            sE = atr.tile([P, T], BF16, tag="sE", bufs=8, name="sE")
                            nc.scalar.activation(
                                sE[:, m:], psts[i][:, m:],
                                mybir.ActivationFunctionType.Exp, scale=SCALE)
                            sEs.append(sE)
                        for i, h in enumerate(heads):
                            # row-sum accumulation on DVE (PE stays free)
                            if kk == 0:
                                nc.vector.tensor_copy(saccs[i][:], sEs[i][:])
                            else:
                                nc.vector.tensor_add(saccs[i][:, m:], saccs[i][:, m:],
                                                     sEs[i][:, m:])
                            nc.tensor.matmul(pavs[i][:, m:], v_sb[:, kk, h, :],
                                             sEs[i][:, m:],
                                             start=(kk == 0), stop=(kk == nkt - 1))
                    # normalize both heads
                    for i, h in enumerate(heads):
                        ps1 = aps.tile([1, T], F32, tag="pst", bufs=4, name="ps1")
                        nc.tensor.matmul(ps1[:], ones_colf[:], saccs[i][:],
                                         start=True, stop=True)
                        s1_sb = atr.tile([1, T], F32, tag="s1_sb", bufs=2, name="s1_sb")
                        nc.scalar.copy(s1_sb[:], ps1[:])
                        pbc = aps.tile([P, T], F32, tag="pst", bufs=4, name="pbc")
                        nc.tensor.matmul(pbc[:], ones_row[:], s1_sb[:],
                                         start=True, stop=True)
                        rbc = atr.tile([P, T], F32, tag="rbc", bufs=2, name="rbc")
                        nc.vector.reciprocal_approx_fast(out=rbc[:], in_=pbc[:])
                        nc.vector.tensor_mul(attnT[:, h, g * T:(g + 1) * T],
                                             pavs[i][:], rbc[:])
                # output projection for this q block (2 psum banks, 2 passes)
                for tt in range(g * GQ, (g + 1) * GQ):
                    for dcp in range(2):
                        pos = [aps.tile([P, T], F32, tag="po", bufs=2,
                                        name=f"po{dc}") for dc in range(2)]
                        for h in range(H):
                            for dc in range(2):
                                nc.tensor.matmul(
                                    pos[dc][:], attnT[:, h, tt * P:(tt + 1) * P],
                                    wo_sb[:, h, (2 * dcp + dc) * T:(2 * dcp + dc + 1) * T],
                                    start=(h == 0), stop=(h == H - 1))
                        for dc in range(2):
                            ot = ostg.tile([P, T], F32, tag="ot", bufs=4, name="ot")
                            nc.vector.tensor_copy(ot[:], pos[dc][:])
                            nc.sync.dma_start(
                                out_d.ap()[tt * P:(tt + 1) * P,
                                           (2 * dcp + dc) * T:(2 * dcp + dc + 1) * T],
                                ot[:])
            att2.close()
    nc.compile()
    return nc


def _get_nc():
    if "nc" not in _CACHE:
        _CACHE["nc"] = _build()
    return _CACHE["nc"]


def kernel(x, freqs_cos, freqs_sin, Wq_down, Wq_up, Wq_rope, Wkv_down,
           Wk_up, Wv_up, Wk_rope, Wo):
    nc = _get_nc()
    f32 = np.float32
    x = np.ascontiguousarray(x, f32)
    in_maps = []
    for c in range(8):
        b, hg = c // 4, c % 4
        qs = slice(hg * H * DH, (hg + 1) * H * DH)     # head-dim cols (512)
        rs = slice(hg * H * DR, (hg + 1) * H * DR)     # rope cols (256)
        in_maps.append({
            "x_b": x[b],
            "freqs_cos": np.ascontiguousarray(freqs_cos, f32),
            "freqs_sin": np.ascontiguousarray(freqs_sin, f32),
            "Wq_down": np.ascontiguousarray(Wq_down, f32),
            "Wkv_down": np.ascontiguousarray(Wkv_down, f32),
            "Wq_up_s": np.ascontiguousarray(Wq_up[:, qs], f32),
            "Wk_up_s": np.ascontiguousarray(Wk_up[:, qs], f32),
            "Wv_up_s": np.ascontiguousarray(Wv_up[:, qs], f32),
            "Wq_rope_s": np.ascontiguousarray(Wq_rope[:, rs], f32),
            "Wk_rope_s": np.ascontiguousarray(Wk_rope[:, rs], f32),
            "Wo_s": np.ascontiguousarray(Wo[qs, :], f32),
        })
    res = run_bass_kernel_spmd(nc, in_maps, core_ids=list(range(8)))
    out = np.zeros((2, S, D), np.float32)
    for c in range(8):
        out[c // 4] += res.results[c]["out_partial"]
    return out


# revision 16
# speedup vs baseline: 1.0257x; 1.0257x over previous
"""MLA (multi-head latent attention) prefill kernel for Trainium2, 8 NeuronCores.

Sharding: 8 cores = 2 batches x 4 head-groups (4 heads each). Each core
computes a partial output (its heads' contribution through Wo); host sums
the 4 partials per batch.

Per-core dataflow (feature-major "transposed" layout so weights serve
directly as PE stationary operands):
  x -> xT (PE transpose) -> c_qT/c_kvT (down proj) -> qT/kT (up proj,
  + rope computed in natural layout then transposed) ; v in natural layout.
  Scores computed transposed sT[k,q] = kT.T @ qT, causal-tiled; exp on ACT;
  row sums via DVE accumulate + ones-matmul partition reduce; av as
  avT[d,q] = v.T @ sExpT; normalize; out = attnT.T @ Wo_slice.
All matmuls bf16 with fp32 PSUM accumulation.
"""

import numpy as np
from contextlib import ExitStack

import concourse.bass as bass
import concourse.mybir as mybir
import concourse.tile as tile
from concourse import bacc
from concourse.masks import make_identity
from concourse.bass_utils import run_bass_kernel_spmd

P = 128
S = 2048          # sequence length
D = 2048          # model dim
DC = 512          # latent dim
H = 4             # heads per core
DH = 128          # head dim (nope)
DR = 64           # rope dim per head
T = 512           # token tile
NT = S // T       # 4 token tiles
KC = D // P       # 16 contraction chunks over model dim
NKT = S // P      # 16 key tiles
GQ = T // P       # 4 q-subtiles per token tile
SCALE = 1.0 / float(np.sqrt(DH))

F32 = mybir.dt.float32
BF16 = mybir.dt.bfloat16

_CACHE = {}


def _build():
    nc = bacc.Bacc(None, target_bir_lowering=False)

    # ---- DRAM I/O (per-core shard) ----
    x_d = nc.dram_tensor("x_b", [S, D], F32, kind="ExternalInput")
    cos_d = nc.dram_tensor("freqs_cos", [S, DR // 2], F32, kind="ExternalInput")
    sin_d = nc.dram_tensor("freqs_sin", [S, DR // 2], F32, kind="ExternalInput")
    wqd_d = nc.dram_tensor("Wq_down", [D, DC], F32, kind="ExternalInput")
    wkvd_d = nc.dram_tensor("Wkv_down", [D, DC], F32, kind="ExternalInput")
    wqu_d = nc.dram_tensor("Wq_up_s", [DC, H * DH], F32, kind="ExternalInput")
    wku_d = nc.dram_tensor("Wk_up_s", [DC, H * DH], F32, kind="ExternalInput")
    wvu_d = nc.dram_tensor("Wv_up_s", [DC, H * DH], F32, kind="ExternalInput")
    wqr_d = nc.dram_tensor("Wq_rope_s", [D, H * DR], F32, kind="ExternalInput")
    wkr_d = nc.dram_tensor("Wk_rope_s", [D, H * DR], F32, kind="ExternalInput")
    wo_d = nc.dram_tensor("Wo_s", [H * DH, D], F32, kind="ExternalInput")
    out_d = nc.dram_tensor("out_partial", [S, D], F32, kind="ExternalOutput")

    with tile.TileContext(nc) as tc, ExitStack() as top:
        const = top.enter_context(tc.tile_pool(name="const", bufs=1))
        actp = top.enter_context(tc.tile_pool(name="actp", bufs=1))

        # ---- constants ----
        ident = const.tile([P, P], BF16, tag="ident")
        make_identity(nc, ident[:])
        ones_col = const.tile([P, 1], BF16, tag="ones_col")
        nc.vector.memset(ones_col[:], 1.0)
        ones_colf = const.tile([P, 1], F32, tag="ones_colf")
        nc.vector.memset(ones_colf[:], 1.0)
        ones_row = const.tile([1, P], F32, tag="ones_row")
        nc.vector.memset(ones_row[:], 1.0)
        cos_sb = const.tile([P, NKT, DR // 2], F32, tag="cos")
        sin_sb = const.tile([P, NKT, DR // 2], F32, tag="sin")
        nc.sync.dma_start(cos_sb[:], cos_d.ap().rearrange("(n p) f -> p n f", p=P))
        nc.sync.dma_start(sin_sb[:], sin_d.ap().rearrange("(n p) f -> p n f", p=P))
        masks = []
        for i in range(4):
            m = const.tile([P, T], BF16, tag=f"mask{i}", name=f"mask{i}")
            nc.gpsimd.memset(m[:], 0.0)
            # keep (0) where qcol >= p + 128*i, else -30000
            nc.gpsimd.affine_select(
                out=m[:], in_=m[:], compare_op=mybir.AluOpType.is_ge,
                fill=-30000.0, base=-(P * i), channel_multiplier=-1,
                pattern=[[1, T]],
            )
            masks.append(m)

        # ---- persistent activations ----
        q_nT = actp.tile([P, H, S], BF16, tag="q_nT")
        k_nT = actp.tile([P, H, S], BF16, tag="k_nT")
        q_rT = actp.tile([P, 2, S], BF16, tag="q_rT")   # [ (h%2)*64+r, h//2, t ]
        k_rT = actp.tile([P, 2, S], BF16, tag="k_rT")
        v_sb = actp.tile([P, NKT, H, DH], BF16, tag="v_sb")

        # ================= Phase 1+2+3: projections =================
        with ExitStack() as proj:
            wts = proj.enter_context(tc.tile_pool(name="wts", bufs=1))
            stg = proj.enter_context(tc.tile_pool(name="stg", bufs=2))
            ptr = proj.enter_context(tc.tile_pool(name="ptr", bufs=1))
            pps = proj.enter_context(tc.tile_pool(name="pps", bufs=1, space="PSUM"))

            # ---- load + cast weights (except Wo) ----
            w_dn = wts.tile([P, KC, 2 * DC], BF16, tag="w_dn")     # [:, :, :512]=q, [:, :, 512:]=kv
            w_rp = wts.tile([P, KC, 2 * H * DR], BF16, tag="w_rp")  # q ropes | k ropes
            w_qu = wts.tile([P, DC // P, H * DH], BF16, tag="w_qu")
            w_ku = wts.tile([P, DC // P, H * DH], BF16, tag="w_ku")
            w_vu = wts.tile([P, DC // P, H * DH], BF16, tag="w_vu")

            def load_x_sub(j, s4, xT):
                # load 128 x rows, cast to bf16, PE-transpose into xT chunks
                st = stg.tile([P, D], F32, tag="stage", name="xst")
                nc.sync.dma_start(st[:], x_d.ap()[(j * GQ + s4) * P:(j * GQ + s4 + 1) * P, :])
                xbf = ptr.tile([P, D], BF16, tag="xbf", bufs=2, name="xbf")
                nc.vector.tensor_copy(xbf[:], st[:])
                for k in range(KC):
                    tp = pps.tile([P, P], BF16, tag="tp", bufs=2, name="tp")
                    nc.tensor.transpose(tp[:], xbf[:, k * P:(k + 1) * P], ident[:])
                    nc.scalar.copy(xT[:, k, s4 * P:(s4 + 1) * P], tp[:])

            xT0 = ptr.tile([P, KC, T], BF16, tag="xT", bufs=2, name="xT0")
            for s4 in range(GQ):
                load_x_sub(0, s4, xT0)

            wqd_r = wqd_d.ap().rearrange("(kc p) c -> p kc c", p=P)
            wkvd_r = wkvd_d.ap().rearrange("(kc p) c -> p kc c", p=P)
            for kc4 in range(0, KC, 4):
                for wsrc, dst_off in ((wqd_r, 0), (wkvd_r, DC)):
                    st = stg.tile([P, 4, DC], F32, tag="stage", name="wst")
                    nc.sync.dma_start(st[:], wsrc[:, kc4:kc4 + 4, :])
                    nc.vector.tensor_copy(
                        w_dn[:, kc4:kc4 + 4, dst_off:dst_off + DC], st[:])
            wqr_r = wqr_d.ap().rearrange("(kc p) r -> p kc r", p=P)
            wkr_r = wkr_d.ap().rearrange("(kc p) r -> p kc r", p=P)
            for kc8 in range(0, KC, 8):
                for wsrc, dst_off in ((wqr_r, 0), (wkr_r, H * DR)):
                    st = stg.tile([P, 8, H * DR], F32, tag="stage", name="wst2")
                    nc.sync.dma_start(st[:], wsrc[:, kc8:kc8 + 8, :])
                    nc.vector.tensor_copy(
                        w_rp[:, kc8:kc8 + 8, dst_off:dst_off + H * DR], st[:])
            for wd, wt in ((wqu_d, w_qu), (wku_d, w_ku), (wvu_d, w_vu)):
                st = stg.tile([P, DC // P, H * DH], F32, tag="stage", name="wst3")
                nc.sync.dma_start(st[:], wd.ap().rearrange("(cc p) d -> p cc d", p=P))
                nc.vector.tensor_copy(wt[:], st[:])

            xT = xT0
            for j in range(NT):  # token tiles
                xT_next = (ptr.tile([P, KC, T], BF16, tag="xT", bufs=2, name="xTn")
                           if j + 1 < NT else None)

                # ---- P2a: down projections c_qT, c_kvT ----
                # next tile's transposes interleaved between MM blocks (HAM warmth)
                c_qT = ptr.tile([P, DC // P, T], BF16, tag="c_qT", name="c_qT")
                c_kvT = ptr.tile([P, DC // P, T], BF16, tag="c_kvT", name="c_kvT")
                blk = 0
                for fam, (dst, woff) in enumerate(((c_qT, 0), (c_kvT, DC))):
                    for half in range(2):
                        pa = pps.tile([P, T], F32, tag="acc", bufs=4, name="pa")
                        pb = pps.tile([P, T], F32, tag="acc", bufs=4, name="pb")
                        for k in range(KC):
                            nc.tensor.matmul(
                                pa[:], w_dn[:, k, woff + half * 256: woff + half * 256 + P],
                                xT[:, k, :], start=(k == 0), stop=(k == KC - 1))
                            nc.tensor.matmul(
                                pb[:], w_dn[:, k, woff + half * 256 + P: woff + half * 256 + 2 * P],
                                xT[:, k, :], start=(k == 0), stop=(k == KC - 1))
                        nc.vector.tensor_copy(dst[:, 2 * half, :], pa[:])
                        nc.vector.tensor_copy(dst[:, 2 * half + 1, :], pb[:])
                        if xT_next is not None:
                            load_x_sub(j + 1, blk, xT_next)
                        blk += 1

                # ---- P2b: rope projections (natural layout), apply rope, transpose ----
                for s4 in range(GQ):
                    pr = pps.tile([P, T], F32, tag="acc", bufs=4, name="pr")
                    for k in range(KC):
                        nc.tensor.matmul(
                            pr[:], xT[:, k, s4 * P:(s4 + 1) * P], w_rp[:, k, :],
                            start=(k == 0), stop=(k == KC - 1))
                    # cos/sin replicated x8 across (2 fams x 4 heads)
                    crep = ptr.tile([P, 8, DR // 2], F32, tag="crep", bufs=2, name="crep")
                    srep = ptr.tile([P, 8, DR // 2], F32, tag="srep", bufs=2, name="srep")
                    for r in range(8):
                        nc.scalar.copy(crep[:, r, :], cos_sb[:, j * GQ + s4, :])
                        nc.scalar.copy(srep[:, r, :], sin_sb[:, j * GQ + s4, :])
                    # rope apply: out_r = xr*c - xi*s ; out_i = xr*s + xi*c
                    xr = pr[:, 0:2 * H * DR:2]
                    xi = pr[:, 1:2 * H * DR:2]
                    t1 = ptr.tile([P, H * DR], F32, tag="t1", bufs=2, name="t1")
                    t2 = ptr.tile([P, H * DR], F32, tag="t2", bufs=2, name="t2")
                    rap = ptr.tile([P, 2 * H * DR], BF16, tag="rap", bufs=2, name="rap")
                    nc.vector.tensor_mul(t1[:], xr, crep[:].rearrange("p a b -> p (a b)"))
                    nc.vector.tensor_mul(t2[:], xi, srep[:].rearrange("p a b -> p (a b)"))
                    nc.vector.tensor_sub(rap[:, 0:2 * H * DR:2], t1[:], t2[:])
                    nc.vector.tensor_mul(t1[:], xr, srep[:].rearrange("p a b -> p (a b)"))
                    nc.vector.tensor_mul(t2[:], xi, crep[:].rearrange("p a b -> p (a b)"))
                    nc.vector.tensor_add(rap[:, 1:2 * H * DR:2], t1[:], t2[:])
                    # transpose rope cols to ropeT layout
                    for rc in range(4):
                        tp = pps.tile([P, P], BF16, tag="tp", bufs=2, name="tpr")
                        nc.tensor.transpose(tp[:], rap[:, rc * P:(rc + 1) * P], ident[:])
                        dst = q_rT if rc < 2 else k_rT
                        nc.scalar.copy(
                            dst[:, rc % 2, (j * GQ + s4) * P:(j * GQ + s4 + 1) * P], tp[:])

                # ---- P3: up projections qT, kT (transposed), v (natural) ----
                for h in range(H):
                    pq = pps.tile([P, T], F32, tag="acc", bufs=4, name="pq")
                    pk = pps.tile([P, T], F32, tag="acc", bufs=4, name="pk")
                    for cc in range(DC // P):
                        nc.tensor.matmul(pq[:], w_qu[:, cc, h * DH:(h + 1) * DH],
                                         c_qT[:, cc, :], start=(cc == 0), stop=(cc == 3))
                        nc.tensor.matmul(pk[:], w_ku[:, cc, h * DH:(h + 1) * DH],
                                         c_kvT[:, cc, :], start=(cc == 0), stop=(cc == 3))
                    nc.vector.tensor_copy(q_nT[:, h, j * T:(j + 1) * T], pq[:])
                    nc.vector.tensor_copy(k_nT[:, h, j * T:(j + 1) * T], pk[:])
                for s4 in range(GQ):
                    pv = pps.tile([P, H * DH], F32, tag="acc", bufs=4, name="pv")
                    for cc in range(DC // P):
                        nc.tensor.matmul(pv[:], c_kvT[:, cc, s4 * P:(s4 + 1) * P],
                                         w_vu[:, cc, :], start=(cc == 0), stop=(cc == 3))
                    for h in range(H):
                        nc.vector.tensor_copy(
                            v_sb[:, j * GQ + s4, h, :], pv[:, h * DH:(h + 1) * DH])
                xT = xT_next

        # ================= Phase 4: attention =================
        attnT = top.enter_context(tc.tile_pool(name="attnTp", bufs=1)).tile(
            [P, H, S], BF16, tag="attnT")
        with ExitStack() as att:
            atr = att.enter_context(tc.tile_pool(name="atr", bufs=1))
            aps = att.enter_context(tc.tile_pool(name="aps", bufs=1, space="PSUM"))
            for hp in range(2):
                heads = (2 * hp, 2 * hp + 1)
                for g in range(S // T):
                    pavs = [aps.tile([P, T], F32, tag="pav", bufs=2, name="pav")
                            for _ in range(2)]
                    ps1s = [aps.tile([1, T], F32, tag="ps1", bufs=2, name="ps1")
                            for _ in range(2)]
                    nkt = GQ * g + GQ
                    for kk in range(nkt):
                        # diagonal narrowing: cols < m are fully masked
                        di = kk - GQ * g
                        m = max(di, 0) * P
                        qs = slice(g * T + m, (g + 1) * T)
                        psts = [aps.tile([P, T], F32, tag="pst", bufs=4, name="pst")
                                for _ in range(2)]
                        for i, h in enumerate(heads):
                            nc.tensor.matmul(
                                psts[i][:, m:], k_nT[:, h, kk * P:(kk + 1) * P],
                                q_nT[:, h, qs], start=True, stop=False)
                        for i, h in enumerate(heads):
                            rb = i * DR
                            nc.tensor.matmul(
                                psts[i][:, m:], k_rT[rb:rb + DR, hp, kk * P:(kk + 1) * P],
                                q_rT[rb:rb + DR, hp, qs],
                                start=False, stop=True)
                        sEs = []
                        for i, h in enumerate(heads):
                            if di >= 0:
                                nc.vector.tensor_add(
                                    psts[i][:, m:], psts[i][:, m:], masks[di][:, m:])
                            sE = atr.tile([P, T], BF16, tag="sE", bufs=8, name="sE")
                            nc.scalar.activation(
                                sE[:, m:], psts[i][:, m:],
                                mybir.ActivationFunctionType.Exp, scale=SCALE)
                            sEs.append(sE)
                        for i, h in enumerate(heads):
                            nc.tensor.matmul(ps1s[i][:, m:], ones_col[:], sEs[i][:, m:],
                                             start=(kk == 0), stop=(kk == nkt - 1))
                            nc.tensor.matmul(pavs[i][:, m:], v_sb[:, kk, h, :],
                                             sEs[i][:, m:],
                                             start=(kk == 0), stop=(kk == nkt - 1))
                    # normalize both heads
                    for i, h in enumerate(heads):
                        s1_sb = atr.tile([1, T], F32, tag="s1_sb", bufs=2, name="s1_sb")
                        nc.scalar.copy(s1_sb[:], ps1s[i][:])
                        pbc = aps.tile([P, T], F32, tag="pst", bufs=4, name="pbc")
                        nc.tensor.matmul(pbc[:], ones_row[:], s1_sb[:],
                                         start=True, stop=True)
                        rbc = atr.tile([P, T], F32, tag="rbc", bufs=2, name="rbc")
                        nc.vector.reciprocal_approx_fast(out=rbc[:], in_=pbc[:])
                        nc.vector.tensor_mul(attnT[:, h, g * T:(g + 1) * T],
                                             pavs[i][:], rbc[:])
        # ================= Phase 5: output projection =================
        with ExitStack() as outp:
            wop = outp.enter_context(tc.tile_pool(name="wop", bufs=1))
            ostg = outp.enter_context(tc.tile_pool(name="ostg", bufs=2))
            ops = outp.enter_context(tc.tile_pool(name="ops", bufs=1, space="PSUM"))
            wo_sb = wop.tile([P, H, D], BF16, tag="wo_sb")
            wo_r = wo_d.ap().rearrange("(hh p) dd -> p hh dd", p=P)
            for h2 in range(0, H, 2):
                st = ostg.tile([P, 2, D], F32, tag="ostage", name="wost")
                nc.sync.dma_start(st[:], wo_r[:, h2:h2 + 2, :])
                nc.vector.tensor_copy(wo_sb[:, h2:h2 + 2, :], st[:])
            for tt in range(S // P):
                pos = [ops.tile([P, T], F32, tag="po", bufs=8, name=f"po{dc}")
                       for dc in range(4)]
                for h in range(H):
                    for dc in range(4):
                        nc.tensor.matmul(
                            pos[dc][:], attnT[:, h, tt * P:(tt + 1) * P],
                            wo_sb[:, h, dc * T:(dc + 1) * T],
                            start=(h == 0), stop=(h == H - 1))
                for dc in range(4):
                    ot = ostg.tile([P, T], F32, tag="ot", bufs=8, name="ot")
                    nc.vector.tensor_copy(ot[:], pos[dc][:])
                    nc.sync.dma_start(
                        out_d.ap()[tt * P:(tt + 1) * P, dc * T:(dc + 1) * T], ot[:])

    nc.compile()
    return nc


def _get_nc():
    if "nc" not in _CACHE:
        _CACHE["nc"] = _build()
    return _CACHE["nc"]


def kernel(x, freqs_cos, freqs_sin, Wq_down, Wq_up, Wq_rope, Wkv_down,
           Wk_up, Wv_up, Wk_rope, Wo):
    nc = _get_nc()
    f32 = np.float32
    x = np.ascontiguousarray(x, f32)
    in_maps = []
    for c in range(8):
        b, hg = c // 4, c % 4
        qs = slice(hg * H * DH, (hg + 1) * H * DH)     # head-dim cols (512)
        rs = slice(hg * H * DR, (hg + 1) * H * DR)     # rope cols (256)
        in_maps.append({
            "x_b": x[b],
            "freqs_cos": np.ascontiguousarray(freqs_cos, f32),
            "freqs_sin": np.ascontiguousarray(freqs_sin, f32),
            "Wq_down": np.ascontiguousarray(Wq_down, f32),
            "Wkv_down": np.ascontiguousarray(Wkv_down, f32),
            "Wq_up_s": np.ascontiguousarray(Wq_up[:, qs], f32),
            "Wk_up_s": np.ascontiguousarray(Wk_up[:, qs], f32),
            "Wv_up_s": np.ascontiguousarray(Wv_up[:, qs], f32),
            "Wq_rope_s": np.ascontiguousarray(Wq_rope[:, rs], f32),
            "Wk_rope_s": np.ascontiguousarray(Wk_rope[:, rs], f32),
            "Wo_s": np.ascontiguousarray(Wo[qs, :], f32),
        })
    res = run_bass_kernel_spmd(nc, in_maps, core_ids=list(range(8)))
    out = np.zeros((2, S, D), np.float32)
    for c in range(8):
        out[c // 4] += res.results[c]["out_partial"]
    return out
